# revision 28
# baseline (speedup 1.0000x reference)
"""Trainium2 Bass kernel: GQA attention block (S=2048, HID=4096, 32 q heads /
8 kv heads, head dim 128, RoPE, causal), tensor-parallel over heads on 8
NeuronCores.

Sharding: core c owns q heads [4c..4c+4) and kv head c. wq/wk/wv are sharded
on their output dim, wo on its input dim; each core computes a partial
y_c = o_c @ wo_c.T and the host sums the 8 partials (the "all-reduce").

Everything on-device runs in a transposed [feature, seq] layout so that every
matmul streams wide moving operands:
  qT = wqT-blocks.T @ xT-blocks        (accumulated over K in PSUM)
  scoresT[kk, s] = kT-block.T @ qT     (per 128-key block, 512-seq chunk)
  probsT = exp(scoresT * 1/sqrt(128)), causal via narrowed matmuls + one
           triangular affine_select per diagonal block
  oT += vnat-block.T @ probsT          (accumulated over key blocks)
  denom: probsT accumulated over key blocks on DVE, then one ones-stationary
         matmul per (chunk, head) -> den broadcast across all 128 partitions
  yT = woT-blocks.T @ (oT * 1/denom)

v15 structure: phases interleaved per 512-seq chunk
  p1(0) p1(1) p2(0) p1(2) p2(1) p3(0) p1(3) p2(2) p3(1) p2(3) p3(2) p3(3)
so RoPE / softmax / normalize chains on ACT/DVE always overlap PE matmuls of
a neighboring phase segment. PSUM tags are partitioned so phases never wait
on each other's bank drains: "acc"=6 (p1 accumulators + p2 o_ps), "cyc"=2
(warmup, v-transposes, s_ps, den_ps, y_ps).

RoPE uses a head-dim permutation (even dims first, odd dims second, folded
into the wq/wk rows on the host) so rotation pairs are the two partition
halves; the cross-half operands come from two partition-shifting ACT copies.

bf16 storage/matmuls by default: full PE rate, FWL weight loads (~116ns,
hidden under the 256ns streams), half the DMA; rel err ~5e-3 vs 2e-2 budget.
"""

import os
import sys

import numpy as np

for _p in (
    "/root/.axon_site",
    "/root/.axon_site/_ro/trn_rl_repo",
    "/root/.axon_site/_ro/pypackages",
    "/opt/trn_rl_repo",
):
    if os.path.isdir(_p) and _p not in sys.path:
        sys.path.append(_p)

import concourse.bacc as bacc  # noqa: E402
import concourse.mybir as mybir  # noqa: E402
from concourse import bass_utils  # noqa: E402
from concourse.tile import TileContext  # noqa: E402

F32 = mybir.dt.float32
F32R = mybir.dt.float32r

N_CORES = 8
SEQ = 2048
HID = 4096
NQ = 32
NKV = 8
HD = 128
THETA = 500000.0

HQ = NQ // N_CORES  # 4 q heads per core
QC = HQ * HD  # 512: per-core q feature slice
KB = SEQ // 128  # 16 key blocks
NKBLK = HID // 128  # 32 contraction blocks for the projections
NCHUNK = SEQ // 512  # 4 sequence chunks of 512
SCALE = 1.0 / float(np.sqrt(HD))

# "bf16": bf16 storage/matmuls (fastest: FWL weight loads, ~2x less DMA)
# "f32r": fp32 storage, full-rate fp32r matmuls (~1.5e-4 matmul rel err)
# "f32":  plain fp32 matmuls (4 cycles/row, safest numerics)
MODE = "bf16"


def _mode_dt():
    return {"f32r": F32R, "f32": F32, "bf16": mybir.dt.bfloat16}[MODE]


def _build_body(tc, sb, sbw, ps, mdt):
    nc = tc.nc

    xT = nc.dram_tensor("xT", (HID, SEQ), mdt, kind="ExternalInput").ap()
    wq_sb_d = nc.dram_tensor("wq_sb", (128, NKBLK * QC), mdt, kind="ExternalInput").ap()
    wk_sb_d = nc.dram_tensor("wk_sb", (128, NKBLK * HD), mdt, kind="ExternalInput").ap()
    wv_sb_d = nc.dram_tensor("wv_sb", (128, NKBLK * HD), mdt, kind="ExternalInput").ap()
    wo_sb_d = nc.dram_tensor("wo_sb", (128, 4 * HID), mdt, kind="ExternalInput").ap()
    ones_d = nc.dram_tensor("ones_in", (128, 128), mdt, kind="ExternalInput").ap()
    cc_d = nc.dram_tensor("cc", (HD, SEQ), mdt, kind="ExternalInput").ap()
    ss_d = nc.dram_tensor("ss", (HD, SEQ), mdt, kind="ExternalInput").ap()
    yT_d = nc.dram_tensor("yT", (HID, SEQ), F32, kind="ExternalOutput").ap()
    dscr = nc.dram_tensor("den_scratch", (1, 128), F32).ap()

    # --- persistent SBUF tiles ---
    ones = sb.tile([128, 128], mdt, name="ones")
    nc.sync.dma_start(ones[:], ones_d[:])

    # PE warmup: ~4us of dummy matmuls on a vector-memset tile so the HAM
    # clock gate opens before the first real matmul; the result is kept
    # alive by a tiny DMA into the scratch tensor.
    warm_in = sb.tile([128, 128], F32, name="warm_in")
    nc.vector.memset(warm_in[:], 0.5)
    warm_ps = ps.tile([128, 128], F32, tag="cyc", bufs=2, name="warm_ps")
    for wi in range(16):
        nc.tensor.matmul(warm_ps[:], warm_in[:], warm_in[:], start=(wi == 0), stop=(wi == 15))
    warm_sb = sbw.tile([1, 128], F32, tag="warm", bufs=1, name="warm_sb")
    nc.vector.tensor_copy(warm_sb[0:1, :], warm_ps[0:1, :])
    nc.sync.dma_start(dscr[0:1, 0:128], warm_sb[0:1, :])

    wq_t = sb.tile([128, NKBLK * QC], mdt, name="wq_t")
    wk_t = sb.tile([128, NKBLK * HD], mdt, name="wk_t")
    wv_t = sb.tile([128, NKBLK * HD], mdt, name="wv_t")
    wo_t = sb.tile([128, 4 * HID], mdt, name="wo_t")

    qT = [sb.tile([128, SEQ], mdt, name=f"qT{h}") for h in range(HQ)]
    kT = sb.tile([128, SEQ], mdt, name="kT")
    vnat = sb.tile([128, KB * 128], mdt, name="vnat")

    # =================== phase 1: QKV projections + RoPE ===================
    # RoPE is split: phase1 emits only the PSUM drain copies (prompt bank
    # release); the rope math is a closure the caller emits later, so it runs
    # on ACT/DVE under a p1/p3 matmul segment instead of colliding with
    # phase2's exp stream. All math in bf16 for the DVE fast modes.
    def rope_math(dst, cpy, cct, sst, s0):
        """dst[:, s0:s0+512] = rope(cpy); partition rows 0:64 hold the even
        rope dims, 64:128 the odd ones (host permuted the weight rows)."""
        sw = sbw.tile([128, 512], mdt, tag="ropetmp", bufs=8, name="sw")
        nc.scalar.copy(sw[0:64, :], cpy[64:128, :])
        nc.scalar.copy(sw[64:128, :], cpy[0:64, :])
        m1 = sbw.tile([128, 512], mdt, tag="ropetmp", bufs=8, name="m1")
        m2 = sbw.tile([128, 512], mdt, tag="ropetmp", bufs=8, name="m2")
        nc.vector.tensor_mul(m1[:], cpy[:], cct[:])
        nc.vector.tensor_mul(m2[:], sw[:], sst[:])
        nc.vector.tensor_sub(dst[0:64, s0 : s0 + 512], m1[0:64, :], m2[0:64, :])
        nc.vector.tensor_add(dst[64:128, s0 : s0 + 512], m1[64:128, :], m2[64:128, :])

    def phase1(sc_i):
        s0 = sc_i * 512
        q_ps = [ps.tile([128, 512], F32, tag="acc", bufs=6, name=f"q_ps{h}") for h in range(HQ)]
        k_ps = ps.tile([128, 512], F32, tag="acc", bufs=6, name="k_ps")
        v_ps = ps.tile([128, 512], F32, tag="acc", bufs=6, name="v_ps")
        for k in range(NKBLK):
            if sc_i == 0 and k % 4 == 0:
                # stream weights in 4-k-block pieces so the first matmuls can
                # start early while keeping the DMA instruction count low
                nc.sync.dma_start(wq_t[:, k * QC : (k + 4) * QC], wq_sb_d[:, k * QC : (k + 4) * QC])
                nc.sync.dma_start(wk_t[:, k * HD : (k + 4) * HD], wk_sb_d[:, k * HD : (k + 4) * HD])
                nc.sync.dma_start(wv_t[:, k * HD : (k + 4) * HD], wv_sb_d[:, k * HD : (k + 4) * HD])
            if sc_i == 1 and k % 8 == 0:
                # stage the wo weights during chunk 1; first needed by p3(0)
                p = k // 8
                nc.sync.dma_start(
                    wo_t[:, p * HID : (p + 1) * HID], wo_sb_d[:, p * HID : (p + 1) * HID]
                )
            if k % 4 == 0:
                # one batched DMA per 4 k-blocks keeps the DGE queues shallow
                xt4 = sbw.tile([128, 2048], mdt, tag="xstream", bufs=3, name="xt4")
                nc.sync.dma_start(
                    xt4[:].rearrange("p (f s) -> p f s", f=4),
                    xT[k * 128 : (k + 4) * 128, s0 : s0 + 512].rearrange(
                        "(f p) s -> p f s", f=4
                    ),
                )
            xt = xt4[:, (k % 4) * 512 : (k % 4 + 1) * 512]
            st = k == 0
            sp = k == NKBLK - 1
            for h in range(HQ):
                wsl = wq_t[:, k * QC + h * 128 : k * QC + (h + 1) * 128]
                nc.tensor.matmul(q_ps[h][:], wsl, xt, start=st, stop=sp)
            nc.tensor.matmul(k_ps[:], wk_t[:, k * HD : (k + 1) * HD], xt, start=st, stop=sp)
            nc.tensor.matmul(v_ps[:], wv_t[:, k * HD : (k + 1) * HD], xt, start=st, stop=sp)
        cct = sbw.tile([128, 512], mdt, tag="tbl", bufs=4, name="cct")
        sst = sbw.tile([128, 512], mdt, tag="tbl", bufs=4, name="sst")
        nc.sync.dma_start(cct[:], cc_d[:, s0 : s0 + 512])
        nc.sync.dma_start(sst[:], ss_d[:, s0 : s0 + 512])
        # v first (before the rope drains queue up the engines): PSUM holds
        # the vT chunk [d, s]; PE-transpose 128-blocks into vnat [kk, d].
        # All copies on DVE so the "cyc" banks release promptly.
        vtmp = sbw.tile([128, 512], F32, tag="vtmp", bufs=2, name="vtmp")
        nc.vector.tensor_copy(vtmp[:], v_ps[:])
        for i in range(4):
            j = 4 * sc_i + i
            tp = ps.tile([128, 128], F32, tag="cyc", bufs=2, name="tp")
            nc.tensor.transpose(tp[:], vtmp[:, i * 128 : (i + 1) * 128], ident_for(tc, sb))
            nc.vector.tensor_copy(vnat[:, j * 128 : (j + 1) * 128], tp[:])
        # rope drains: alternate ACT/DVE so the 6 "acc" banks free in ~2.5us
        ropes = []
        for idx, (dst, psrc) in enumerate(
            [(qT[h], q_ps[h]) for h in range(HQ)] + [(kT, k_ps)]
        ):
            cpy = sbw.tile([128, 512], mdt, tag="ropecpy", bufs=10, name="cpy")
            if idx % 2 == 0:
                nc.scalar.copy(cpy[:], psrc[:])
            else:
                nc.vector.tensor_copy(cpy[:], psrc[:])
            ropes.append((dst, cpy))

        def math():
            for dst, cpy in ropes:
                rope_math(dst, cpy, cct, sst, s0)

        return math

    # =================== phase 2: attention ===================
    # One head at a time: o_ps accumulates over key blocks in PSUM; the
    # softmax denominator accumulates the exp tiles over key blocks on DVE,
    # then one ones-stationary matmul broadcasts the per-query sum across all
    # 128 partitions (no DRAM round-trip).
    def phase2_gen(sc_i):
        """Generator: yields after each (head, key-block) iteration so the
        caller can interleave independent phase3 matmuls between the exp
        producer and its AV consumer (the PE queue is in-order)."""
        s0 = sc_i * 512
        jmax = 4 * sc_i + 3
        # den + normalize for head h is emitted after head h+1 has matmuls in
        # the PE queue, so the PE never stalls on the DVE accumulate chain
        pending = []

        def flush_pending():
            while pending:
                o_ps_p, acc_p, h_p = pending.pop(0)
                den_ps = ps.tile([128, 512], F32, tag="acc", bufs=6, name="den_ps")
                nc.tensor.matmul(den_ps[:], ones[:], acc_p[:], start=True, stop=True)
                rec = sbw.tile([128, 512], F32, tag="bcast", bufs=4, name="rec")
                scr = sbw.tile([128, 512], F32, tag="bcast", bufs=4, name="scr")
                nc.vector.reciprocal_approx_accurate(rec[:], den_ps[:], scr[:])
                # normalized attention output, written over the dead qT chunk
                nc.vector.tensor_mul(qT[h_p][:, s0 : s0 + 512], o_ps_p[:], rec[:])

        for h in range(HQ):
            o_ps = ps.tile([128, 512], F32, tag="acc", bufs=6, name="o_ps")
            acc = sbw.tile([128, 512], mdt, tag="den", bufs=5, name="accden")
            for j in range(jmax + 1):
                # causal: columns below s0+off are fully masked for this block
                off = 128 * max(0, j - 4 * sc_i)
                g = j - 4 * sc_i
                s_ps = ps.tile([128, 512], F32, tag="cyc", bufs=2, name="s_ps")
                nc.tensor.matmul(
                    s_ps[:, off:512],
                    kT[:, j * 128 : (j + 1) * 128],
                    qT[h][:, s0 + off : s0 + 512],
                    start=True,
                    stop=True,
                )
                et = sbw.tile([128, 512], mdt, tag="stream", bufs=12, name="et")
                nc.scalar.activation(
                    et[:, off:512], s_ps[:, off:512],
                    mybir.ActivationFunctionType.Exp, scale=SCALE,
                )
                if g >= 0:  # diagonal block: keep keys kk <= s in block
                    nc.gpsimd.affine_select(
                        out=et[:, g * 128 : (g + 1) * 128],
                        in_=et[:, g * 128 : (g + 1) * 128],
                        compare_op=mybir.AluOpType.is_ge,
                        fill=0.0,
                        base=0,
                        pattern=[[1, 128]],
                        channel_multiplier=-1,
                    )
                nc.tensor.matmul(
                    o_ps[:, off:512], vnat[:, j * 128 : (j + 1) * 128],
                    et[:, off:512], start=(j == 0), stop=(j == jmax),
                )
                if j == 0:
                    nc.vector.tensor_copy(acc[:], et[:])
                else:
                    nc.vector.tensor_add(acc[:, off:512], acc[:, off:512], et[:, off:512])
                if j == 1:
                    flush_pending()
                yield
            pending.append((o_ps, acc, h))
        flush_pending()

    def phase2(sc_i):
        for _ in phase2_gen(sc_i):
            pass

    def phase2_rr0():
        """Chunk 0, head-round-robin: with no phase3 chunk available to pair
        against, independence between heads absorbs the exp latency."""
        s0 = 0
        o_ps = {h: ps.tile([128, 512], F32, tag="acc", bufs=6, name=f"o2_{h}") for h in range(HQ)}
        accs = {h: sbw.tile([128, 512], mdt, tag="den", bufs=5, name=f"acc2_{h}") for h in range(HQ)}
        for j in range(4):
            off = 128 * j
            for h in range(HQ):
                s_ps = ps.tile([128, 512], F32, tag="cyc", bufs=2, name="s_ps")
                nc.tensor.matmul(
                    s_ps[:, off:512], kT[:, j * 128 : (j + 1) * 128],
                    qT[h][:, off:512], start=True, stop=True,
                )
                et = sbw.tile([128, 512], mdt, tag="stream", bufs=12, name="et")
                nc.scalar.activation(
                    et[:, off:512], s_ps[:, off:512],
                    mybir.ActivationFunctionType.Exp, scale=SCALE,
                )
                nc.gpsimd.affine_select(
                    out=et[:, j * 128 : (j + 1) * 128],
                    in_=et[:, j * 128 : (j + 1) * 128],
                    compare_op=mybir.AluOpType.is_ge,
                    fill=0.0,
                    base=0,
                    pattern=[[1, 128]],
                    channel_multiplier=-1,
                )
                nc.tensor.matmul(
                    o_ps[h][:, off:512], vnat[:, j * 128 : (j + 1) * 128],
                    et[:, off:512], start=(j == 0), stop=(j == 3),
                )
                if j == 0:
                    nc.vector.tensor_copy(accs[h][:], et[:])
                else:
                    nc.vector.tensor_add(accs[h][:, off:512], accs[h][:, off:512], et[:, off:512])
        for h in range(HQ):
            den_ps = ps.tile([128, 512], F32, tag="acc", bufs=6, name="den_ps")
            nc.tensor.matmul(den_ps[:], ones[:], accs[h][:], start=True, stop=True)
            rec = sbw.tile([128, 512], F32, tag="bcast", bufs=4, name="rec")
            scr = sbw.tile([128, 512], F32, tag="bcast", bufs=4, name="scr")
            nc.vector.reciprocal_approx_accurate(rec[:], den_ps[:], scr[:])
            nc.vector.tensor_mul(qT[h][:, 0:512], o_ps[h][:], rec[:])

    oT = qT  # qT tiles hold the normalized attention output after phase2

    # =================== phase 3: output projection ===================
    def phase3_gen(sc_i):
        s0 = sc_i * 512
        yst = None
        for jb in range(HID // 128):
            if jb % 2 == 0:
                yst = sbw.tile([128, 1024], F32, tag="ystore", bufs=4, name="yst")
            y_ps = ps.tile([128, 512], F32, tag="acc", bufs=6, name="y_ps")
            for cb in range(4):
                nc.tensor.matmul(
                    y_ps[:],
                    wo_t[:, cb * HID + jb * 128 : cb * HID + (jb + 1) * 128],
                    oT[cb][:, s0 : s0 + 512],
                    start=(cb == 0),
                    stop=(cb == 3),
                )
            half = yst[:, (jb % 2) * 512 : (jb % 2 + 1) * 512]
            if jb % 2 == 0:
                nc.vector.tensor_copy(half, y_ps[:])
            else:
                nc.scalar.copy(half, y_ps[:])
                # one batched store per 2 output blocks
                nc.sync.dma_start(
                    yT_d[(jb - 1) * 128 : (jb + 1) * 128, s0 : s0 + 512].rearrange(
                        "(f p) s -> p f s", f=2
                    ),
                    yst[:].rearrange("p (f s) -> p f s", f=2),
                )
            yield

    def phase3(sc_i):
        for _ in phase3_gen(sc_i):
            pass

    _DONE = object()

    def pair(g2, g3, ratio):
        """Interleave emission: `ratio` phase2 iterations, then one phase3
        iteration, until both generators are exhausted."""
        a_done = b_done = False
        while not (a_done and b_done):
            for _ in range(ratio):
                if next(g2, _DONE) is _DONE:
                    a_done = True
            if next(g3, _DONE) is _DONE:
                b_done = True

    # rope math for chunk c must be emitted before phase2(c); it is deferred
    # so it executes under a matmul-heavy segment, not during phase2's exps.
    # p2 chunks are interleaved with p3 chunks (pair) so independent output-
    # projection matmuls absorb the exp latency in the in-order PE queue.
    m0 = phase1(0)
    m0()
    m1 = phase1(1)
    phase2_rr0()
    m1()
    m2 = phase1(2)
    pair(phase2_gen(1), phase3_gen(0), 1)
    m2()
    m3 = phase1(3)
    m3()
    pair(phase2_gen(2), phase3_gen(1), 2)
    pair(phase2_gen(3), phase3_gen(2), 2)
    phase3(3)


_IDENT = {}


def ident_for(tc, sb, dt=F32):
    if "t" not in _IDENT:
        from concourse.masks import make_identity

        ident = sb.tile([128, 128], dt, name="ident")
        make_identity(tc.nc, ident)
        _IDENT["t"] = ident
    return _IDENT["t"]


_NC_CACHE = {}


def _get_nc():
    key = ("v20", MODE)
    if key not in _NC_CACHE:
        _IDENT.clear()
        mdt = _mode_dt()
        nc = bacc.Bacc("TRN2", target_bir_lowering=False, debug=False, num_devices=N_CORES)
        with TileContext(nc) as tc:
            with (
                tc.tile_pool(name="sb", bufs=1) as sb,
                tc.tile_pool(name="sbw", bufs=1) as sbw,
                tc.tile_pool(name="ps", bufs=1, space="PSUM") as ps,
            ):
                _build_body(tc, sb, sbw, ps, mdt)
        nc.compile()
        _NC_CACHE[key] = nc
    return _NC_CACHE[key]


_ROPE_PERM = np.concatenate([np.arange(0, 128, 2), np.arange(1, 128, 2)])


def _rope_tables(start_pos):
    freqs = 1.0 / (THETA ** (np.arange(0, HD, 2, dtype=np.float64) / HD))
    t = np.arange(start_pos, start_pos + SEQ, dtype=np.float64)
    ang = np.outer(t, freqs)  # [SEQ, 64]
    cosT = np.cos(ang).T.astype(np.float32)  # [64, SEQ]
    sinT = np.sin(ang).T.astype(np.float32)
    cc = np.ascontiguousarray(np.concatenate([cosT, cosT], axis=0))
    ss = np.ascontiguousarray(np.concatenate([sinT, sinT], axis=0))
    return cc, ss


def _to_kblock_layout(wT, cwidth):
    """[HID, cwidth] feature-major weight -> [128, NKBLK*cwidth] with k-block
    k at columns [k*cwidth, (k+1)*cwidth)."""
    return np.ascontiguousarray(
        wT.reshape(NKBLK, 128, cwidth).transpose(1, 0, 2).reshape(128, NKBLK * cwidth)
    )


def make_in_maps(x, wq, wk, wv, wo, start_pos):
    import ml_dtypes

    np_mdt = np.float32 if MODE in ("f32r", "f32") else ml_dtypes.bfloat16
    x = np.asarray(x, dtype=np.float32)
    wq = np.asarray(wq, dtype=np.float32)
    wk = np.asarray(wk, dtype=np.float32)
    wv = np.asarray(wv, dtype=np.float32)
    wo = np.asarray(wo, dtype=np.float32)
    sp = int(start_pos)

    xT = np.ascontiguousarray(x.T).astype(np_mdt, copy=False)
    cc, ss = _rope_tables(sp)
    cc = np.ascontiguousarray(cc.astype(np_mdt, copy=False))
    ss = np.ascontiguousarray(ss.astype(np_mdt, copy=False))
    woT = np.ascontiguousarray(wo.T)  # [in=c, out=j]
    ones_in = np.ones((128, 128), dtype=np.float32)

    in_maps = []
    for c in range(N_CORES):
        wq_c = wq[c * QC : (c + 1) * QC, :]  # [512, HID]
        wq_c = wq_c.reshape(HQ, HD, HID)[:, _ROPE_PERM, :].reshape(QC, HID)
        wk_c = wk[c * HD : (c + 1) * HD, :][_ROPE_PERM, :]  # [128, HID]
        wv_c = wv[c * HD : (c + 1) * HD, :]  # [128, HID]
        wq_sbm = _to_kblock_layout(np.ascontiguousarray(wq_c.T), QC)
        wk_sbm = _to_kblock_layout(np.ascontiguousarray(wk_c.T), HD)
        wv_sbm = _to_kblock_layout(np.ascontiguousarray(wv_c.T), HD)
        woT_c = woT[c * QC : (c + 1) * QC, :]  # [512, HID]
        wo_sbm = np.ascontiguousarray(
            woT_c.reshape(4, 128, HID).transpose(1, 0, 2).reshape(128, 4 * HID)
        )
        in_maps.append(
            {
                "xT": xT,
                "wq_sb": wq_sbm.astype(np_mdt, copy=False),
                "wk_sb": wk_sbm.astype(np_mdt, copy=False),
                "wv_sb": wv_sbm.astype(np_mdt, copy=False),
                "wo_sb": wo_sbm.astype(np_mdt, copy=False),
                "ones_in": ones_in.astype(np_mdt, copy=False),
                "cc": cc,
                "ss": ss,
            }
        )
    return in_maps


def _assemble(results):
    acc = results[0]["yT"].astype(np.float32)
    for r in results[1:]:
        acc = acc + r["yT"]
    return np.ascontiguousarray(acc.T)


def _row0_expected(x, wv, wo):
    """Exact y[0]: query 0 attends only key 0, so o[0] is v[0] broadcast over
    the 4 q heads of each kv head; cheap host-side corruption check."""
    v0 = np.asarray(x[0], np.float64) @ np.asarray(wv, np.float64).T  # [1024]
    o0 = np.concatenate([v0[(h // HQ) * HD : (h // HQ + 1) * HD] for h in range(NQ)])
    return o0 @ np.asarray(wo, np.float64).T  # [4096]


def kernel(x, wq, wk, wv, wo, start_pos):
    nc = _get_nc()
    in_maps = make_in_maps(x, wq, wk, wv, wo, start_pos)
    y0 = _row0_expected(x, wv, wo)
    out = None
    for attempt in range(2):
        res = bass_utils.run_bass_kernel_spmd(nc, in_maps, core_ids=list(range(N_CORES)))
        out = _assemble(res.results)
        err0 = float(np.linalg.norm(out[0] - y0) / (np.linalg.norm(y0) + 1e-30))
        if np.isfinite(out).all() and err0 < 2e-2:
            break
        # a wedged device can corrupt a run silently; one retry clears it
    return out
